# revision 27
# baseline (speedup 1.0000x reference)
"""Trainium2 Bass kernel for nn_Bert segment-mean (segment_reduce).

out[b, w, :] = mean(emb[b, st:ed, :]) if (mask != 0 and ed > st) else 0

Full shapes: emb [64, 512, 1024] f32, offsets [64, 400, 2] i32, mask [64, 400] i32.
Data-parallel over batch: 8 rows per core on 8 NeuronCores.

The contraction is out[w, :] = sum_s span[s, w] * emb[s, :] per batch row,
with span[s, w] = scale_w * (st_w <= s < ed_w), scale_w = 1/len_w.

Host-side specialization (all O(B*W*S) int index work; every shipped float
and all float arithmetic stay on device):
  - invalid words (mask == 0 or ed <= st) produce exactly 0; only the ~100
    valid words per row are packed (order preserved), computed, stored and
    scattered back on host.
  - the s axis is packed: only positions covered by a valid span ship
    (~104-146 of 512 per row). Packed positions are contiguous per word, so
    each row splits at a word boundary into a prefix block of <= 128
    positions (one full-width matmul pass) and a tiny suffix (~0-18
    positions, ~0-14 words).
  - the 8 suffixes of a core are pooled into ONE extra matmul pass
    (block-diagonal span), so a core runs 9 tensor passes instead of the
    12-16 a fixed 128-chunking needs.
  - rows are sorted by coverage and grouped into slots; all cores run one
    SPMD program whose per-slot shapes are the max over the 8 rows (one
    per core) assigned to that slot.

Per-core program (R=8 slots):
  slot r:  psum_r[w, n*512:+512] = span0_r[:c0, w].T @ emb_r[:c0, n*512:+512]
  pooled:  psum_p[v, n*512:+512] = spanp[:SC, v].T @ suf[:SC, n*512:+512]
  copies PSUM->fp16 SBUF alternate ScalarE/VectorE; stores trigger from the
  scalar queue right after each copy.

All input DMAs are triggered from the sync queue, stores from the scalar
queue (both HW-DGE; gpsimd's software-DGE serializes onto queue 0 and is
~4x slower). emb ships cov-packed fp16 as a few large transfers (2KB
contiguous lines) so all 16 HW DMA queues engage early.
"""

import os
import sys

for _p in ("/opt/trn_rl_repo", "/root/.axon_site/_ro/trn_rl_repo"):
    if os.path.isdir(_p) and _p not in sys.path:
        sys.path.insert(0, _p)

import numpy as np

import concourse.bacc as bacc
import concourse.mybir as mybir
import concourse.tile as tile
from concourse.bass_utils import run_bass_kernel_spmd

B, S, W, D = 64, 512, 400, 1024
N_CORES = 8
R = B // N_CORES          # batch rows per core (= slots per program)
NW = 512                  # matmul moving width (PSUM bank = 512 fp32)
BP_CAP = 128              # max prefix contraction size (partition dim)

f32 = mybir.dt.float32
fp16 = mybir.dt.float16

# Results of the most recent run, for test harnesses.
LAST_RESULTS = None


def pack_rows(x_bert_offset, x_mask):
    """Per batch row: valid word idx, covered s-positions, packed st/ed/scale.

    Packed positions are the concatenation of the valid spans in order, so
    stp[w] = edp[w-1] and each position belongs to exactly one valid word.
    The row splits at a word boundary: prefix words [0, w1) cover positions
    [0, bp); suffix words [w1, nv) cover [bp, cov), with bp <= 128.
    """
    st = np.asarray(x_bert_offset)[..., 0].astype(np.int64)
    ed = np.asarray(x_bert_offset)[..., 1].astype(np.int64)
    valid = (np.asarray(x_mask) != 0) & (ed > st)
    rows = []
    for b in range(st.shape[0]):
        idx = np.nonzero(valid[b])[0]
        cov = np.zeros(S, bool)
        for w in idx:
            cov[st[b, w]:ed[b, w]] = True
        ci = np.nonzero(cov)[0]
        stp = np.searchsorted(ci, st[b, idx])
        lens = ed[b, idx] - st[b, idx]
        g = {
            "idx": idx, "ci": ci, "stp": stp, "edp": stp + lens,
            "scale": (1.0 / lens).astype(np.float32),
            "cov": len(ci), "nv": len(idx),
        }
        if g["cov"] <= BP_CAP:
            g["w1"], g["bp"] = g["nv"], g["cov"]
        else:
            w1 = int(np.argmax(g["edp"] > BP_CAP))
            g["w1"], g["bp"] = w1, int(g["stp"][w1])
        g["sw"], g["sc"] = g["nv"] - g["w1"], g["cov"] - g["bp"]
        rows.append(g)
    return rows


def assign_slots(rows):
    """Sort rows by coverage, slot r gets ranks [8r, 8r+8) (one per core).

    Returns per-slot maxima: c0 (prefix positions), np_ (prefix words),
    sc (suffix positions), sw (suffix words).
    """
    order = sorted(range(len(rows)), key=lambda b: -rows[b]["cov"])
    perm = [[order[r * N_CORES + c] for r in range(R)] for c in range(N_CORES)]
    mx = lambda key: tuple(
        max(rows[order[r * N_CORES + c]][key] for c in range(N_CORES))
        for r in range(R)
    )
    return perm, mx("bp"), mx("w1"), mx("sc"), mx("sw")


def build_program(c0s, nps, sc_off, sw_off, sct, swt, groups):
    nc = bacc.Bacc("TRN2", target_bir_lowering=False, debug=False)

    emb_d = nc.dram_tensor("emb", [R, 128 * D], fp16, kind="ExternalInput").ap()
    span0_d = nc.dram_tensor(
        "span0", [128, R + 1, 128], fp16, kind="ExternalInput"
    ).ap()
    if sct:
        suf_d = nc.dram_tensor("suf", [sct, D], fp16, kind="ExternalInput").ap()
        outp_d = nc.dram_tensor("outp", [128, D], fp16, kind="ExternalOutput").ap()
    out_d = nc.dram_tensor("out", [R, 128, D], fp16, kind="ExternalOutput").ap()

    with tile.TileContext(nc) as tc:
        with (
            tc.tile_pool(name="emb", bufs=1) as embp,
            tc.tile_pool(name="span", bufs=1) as spanp,
            tc.tile_pool(name="outs", bufs=6) as outp,
            tc.tile_pool(name="psum", bufs=4, space="PSUM") as psump,
        ):
            emb_t = embp.tile([128, R, D], fp16, name="emb_t")
            span0_t = spanp.tile([128, R + 1, 128], fp16, name="span0_t")
            if sct:
                suf_t = embp.tile([128, D], fp16, name="suf_t")
            psums = []

            def mm_pass(r, span_ap, emb_rows):
                ps = psump.tile([128, D], f32, name="ps")
                psums.append(ps)
                for n in range(D // NW):
                    f0 = n * NW
                    nc.tensor.matmul(
                        ps[:, f0 : f0 + NW],
                        span_ap,
                        emb_rows[:, f0 : f0 + NW],
                        start=True,
                        stop=True,
                    )

            # Emission order: consumers right after their producers' DMA
            # triggers; all inputs on the SP ring (~208GB/s observed; the
            # ACT ring is slower and keeps descriptor-gen time for stores),
            # in consumption order so the ring FIFO matches the matmul
            # order. 3 emb groups so late slots aren't gated on one big
            # transfer; the last group is just the final slot.
            nc.sync.dma_start(out=span0_t[:, 0, :], in_=span0_d[:, 0, :])
            nc.sync.dma_start(
                out=emb_t[: c0s[0], 0, :],
                in_=emb_d[0, : c0s[0] * D].rearrange("(p d) -> p d", d=D),
            )
            mm_pass(0, span0_t[: c0s[0], 0, :], emb_t[: c0s[0], 0, :])
            # bulk span includes the pooled-span slot R
            nc.sync.dma_start(out=span0_t[:, 1:, :], in_=span0_d[:, 1:, :])
            for gi, (lo, hi) in enumerate(groups):
                cg = max(c0s[lo:hi])
                nc.sync.dma_start(
                    out=emb_t[:cg, lo:hi, :],
                    in_=emb_d[lo:hi, : cg * D].rearrange("r (p d) -> p r d", d=D),
                )
                if gi == 0 and sct:
                    # pooled-pass inputs right behind the first group so the
                    # pooled pass stays out of the tail
                    nc.sync.dma_start(out=suf_t[:sct, :], in_=suf_d)
                for r in range(lo, hi):
                    mm_pass(r, span0_t[: c0s[r], r, :], emb_t[: c0s[r], r, :])
                if gi == 0 and sct:
                    mm_pass("pool", span0_t[:sct, R, :], suf_t[:sct, :])

            # copies + stores in completion order: ScalarE/VectorE alternate,
            # the pooled copy goes on ScalarE mid-stream. Full-tile [128, D]
            # copies + contiguous-row stores keep the store DMA on the fast
            # HW-DGE path (sliced APs fall back to ~70ns/descriptor inline
            # generation on the sequencer). VectorE can't trigger DMAs, so
            # its rows store via the SP ring (idle after the input stream).
            keys = [0] + list(range(1, 4)) + (["pool"] if sct else []) + list(range(4, R))
            psum_of = dict(zip(keys, psums))
            seq = [0, 1, 2, 3] + (["pool"] if sct else []) + list(range(4, R))
            eng = {0: "sc", 1: "ve", 2: "sc", 3: "ve", "pool": "sc",
                   4: "ve", 5: "sc", 6: "ve", 7: "sc"}
            for r in seq:
                ps = psum_of[r]
                ot = outp.tile([128, D], fp16, name="ot")
                dst = outp_d if r == "pool" else out_d[r]
                e = eng[r]
                if e == "sc":
                    nc.scalar.copy(ot[:], ps[:])
                    nc.scalar.dma_start(out=dst, in_=ot[:])
                else:
                    if e == "ve":
                        nc.vector.tensor_copy(ot[:], ps[:])
                    else:
                        nc.gpsimd.tensor_copy(ot[:], ps[:])
                    # these engines can't HW-trigger DMAs; sync ring is idle
                    nc.sync.dma_start(out=dst, in_=ot[:])

    nc.compile()
    return nc


_PROGRAM_CACHE = {}


def kernel(bert_embedding, x_bert_offset, x_mask, trace=False):
    global LAST_RESULTS
    assert bert_embedding.shape == (B, S, D), bert_embedding.shape
    rows = pack_rows(x_bert_offset, x_mask)
    assert max(g["nv"] for g in rows) <= 128, "over 128 valid words per row"
    assert max(g["sc"] for g in rows) <= 128 and max(g["sw"] for g in rows) <= 128
    perm, c0s, nps, scs, sws = assign_slots(rows)
    assert sum(scs) <= 128 and sum(sws) <= 128, (
        f"pooled suffix overflow: {sum(scs)} positions, {sum(sws)} words"
    )
    sc_off = tuple(int(x) for x in np.cumsum((0,) + scs[:-1]))
    sw_off = tuple(int(x) for x in np.cumsum((0,) + sws[:-1]))
    sct, swt = sum(scs), sum(sws)
    groups = ((1, 4), (4, 6), (6, 7), (7, 8))

    key = (c0s, nps, sc_off, sw_off, sct, swt, groups)
    if key not in _PROGRAM_CACHE:
        _PROGRAM_CACHE.clear()
        _PROGRAM_CACHE[key] = build_program(
            c0s, nps, sc_off, sw_off, sct, swt, groups
        )
    nc = _PROGRAM_CACHE[key]

    emb16 = np.asarray(bert_embedding).astype(np.float16)
    in_maps = []
    for c in range(N_CORES):
        emb_h = np.zeros((R, 128 * D), np.float16)
        span0_h = np.zeros((128, R + 1, 128), np.float16)
        suf_h = np.zeros((max(sct, 1), D), np.float16)
        for r in range(R):
            b = perm[c][r]
            g = rows[b]
            packed = emb16[b, g["ci"]]  # [cov, D]
            emb_h[r, : g["bp"] * D] = packed[: g["bp"]].ravel()
            # prefix span: words [0, w1) x positions [0, bp)
            p = np.arange(g["bp"])
            w1 = g["w1"]
            m = (p[:, None] >= g["stp"][None, :w1]) & (
                p[:, None] < g["edp"][None, :w1]
            )
            span0_h[: g["bp"], r, :w1] = m * g["scale"][None, :w1]
            if g["sc"]:
                o = sc_off[r]
                suf_h[o : o + g["sc"]] = packed[g["bp"] :]
                # suffix span block: positions [bp, cov) x words [w1, nv)
                p = np.arange(g["bp"], g["cov"])
                m = (p[:, None] >= g["stp"][None, w1:]) & (
                    p[:, None] < g["edp"][None, w1:]
                )
                span0_h[o : o + g["sc"], R, sw_off[r] : sw_off[r] + g["sw"]] = (
                    m * g["scale"][None, w1:]
                )
        m = {"emb": emb_h, "span0": span0_h}
        if sct:
            m["suf"] = suf_h
        in_maps.append(m)

    res = run_bass_kernel_spmd(nc, in_maps, list(range(N_CORES)), trace=trace)
    LAST_RESULTS = res
    out = np.zeros((B, W, D), np.float32)
    for c in range(N_CORES):
        packed = res.results[c]["out"]
        poold = res.results[c]["outp"] if sct else None
        for r in range(R):
            b = perm[c][r]
            g = rows[b]
            out[b, g["idx"][: g["w1"]]] = packed[r, : g["w1"]]
            if g["sw"]:
                o = sw_off[r]
                out[b, g["idx"][g["w1"] :]] = poold[o : o + g["sw"]]
    return out


# revision 28
# speedup vs baseline: 1.1303x; 1.1303x over previous
"""Trainium2 Bass kernel for nn_Bert segment-mean (segment_reduce).

out[b, w, :] = mean(emb[b, st:ed, :]) if (mask != 0 and ed > st) else 0

Full shapes: emb [64, 512, 1024] f32, offsets [64, 400, 2] i32, mask [64, 400] i32.
Data-parallel over batch: 8 rows per core on 8 NeuronCores.

The contraction is out[w, :] = sum_s span[s, w] * emb[s, :] per batch row,
with span[s, w] = scale_w * (st_w <= s < ed_w), scale_w = 1/len_w.

Host-side specialization (all O(B*W*S) int index work; every shipped float
and all float arithmetic stay on device):
  - invalid words (mask == 0 or ed <= st) produce exactly 0; only the ~100
    valid words per row are packed (order preserved), computed, stored and
    scattered back on host.
  - the s axis is packed: only positions covered by a valid span ship
    (~104-146 of 512 per row). Packed positions are contiguous per word, so
    each row splits at a word boundary into a prefix block of <= 128
    positions (one full-width matmul pass) and a tiny suffix (~0-18
    positions, ~0-14 words).
  - the 8 suffixes of a core are pooled into ONE extra matmul pass
    (block-diagonal span), so a core runs 9 tensor passes instead of the
    12-16 a fixed 128-chunking needs.
  - rows are sorted by coverage and grouped into slots; all cores run one
    SPMD program whose per-slot shapes are the max over the 8 rows (one
    per core) assigned to that slot.

Per-core program (R=8 slots):
  slot r:  psum_r[w, n*512:+512] = span0_r[:c0, w].T @ emb_r[:c0, n*512:+512]
  pooled:  psum_p[v, n*512:+512] = spanp[:SC, v].T @ suf[:SC, n*512:+512]
  copies PSUM->fp16 SBUF alternate ScalarE/VectorE; stores trigger from the
  scalar queue right after each copy.

All input DMAs are triggered from the sync queue, stores from the scalar
queue (both HW-DGE; gpsimd's software-DGE serializes onto queue 0 and is
~4x slower). emb ships cov-packed fp16 as a few large transfers (2KB
contiguous lines) so all 16 HW DMA queues engage early.
"""

import os
import sys

for _p in ("/opt/trn_rl_repo", "/root/.axon_site/_ro/trn_rl_repo"):
    if os.path.isdir(_p) and _p not in sys.path:
        sys.path.insert(0, _p)

import numpy as np

import concourse.bacc as bacc
import concourse.mybir as mybir
import concourse.tile as tile
from concourse.bass_utils import run_bass_kernel_spmd

B, S, W, D = 64, 512, 400, 1024
N_CORES = 8
R = B // N_CORES          # batch rows per core (= slots per program)
NW = 512                  # matmul moving width (PSUM bank = 512 fp32)
BP_CAP = 128              # max prefix contraction size (partition dim)

f32 = mybir.dt.float32
fp16 = mybir.dt.float16

# Results of the most recent run, for test harnesses.
LAST_RESULTS = None


def pack_rows(x_bert_offset, x_mask):
    """Per batch row: valid word idx, covered s-positions, packed st/ed/scale.

    Packed positions are the concatenation of the valid spans in order, so
    stp[w] = edp[w-1] and each position belongs to exactly one valid word.
    The row splits at a word boundary: prefix words [0, w1) cover positions
    [0, bp); suffix words [w1, nv) cover [bp, cov), with bp <= 128.
    """
    st = np.asarray(x_bert_offset)[..., 0].astype(np.int64)
    ed = np.asarray(x_bert_offset)[..., 1].astype(np.int64)
    valid = (np.asarray(x_mask) != 0) & (ed > st)
    rows = []
    for b in range(st.shape[0]):
        idx = np.nonzero(valid[b])[0]
        cov = np.zeros(S, bool)
        for w in idx:
            cov[st[b, w]:ed[b, w]] = True
        ci = np.nonzero(cov)[0]
        stp = np.searchsorted(ci, st[b, idx])
        lens = ed[b, idx] - st[b, idx]
        g = {
            "idx": idx, "ci": ci, "stp": stp, "edp": stp + lens,
            "scale": (1.0 / lens).astype(np.float32),
            "cov": len(ci), "nv": len(idx),
        }
        if g["cov"] <= BP_CAP:
            g["w1"], g["bp"] = g["nv"], g["cov"]
        else:
            w1 = int(np.argmax(g["edp"] > BP_CAP))
            g["w1"], g["bp"] = w1, int(g["stp"][w1])
        g["sw"], g["sc"] = g["nv"] - g["w1"], g["cov"] - g["bp"]
        rows.append(g)
    return rows


def assign_slots(rows):
    """Sort rows by coverage, slot r gets ranks [8r, 8r+8) (one per core).

    Returns per-slot maxima: c0 (prefix positions), np_ (prefix words),
    sc (suffix positions), sw (suffix words).
    """
    order = sorted(range(len(rows)), key=lambda b: -rows[b]["cov"])
    perm = [[order[r * N_CORES + c] for r in range(R)] for c in range(N_CORES)]
    mx = lambda key: tuple(
        max(rows[order[r * N_CORES + c]][key] for c in range(N_CORES))
        for r in range(R)
    )
    return perm, mx("bp"), mx("w1"), mx("sc"), mx("sw")


def build_program(c0s, nps, sc_off, sw_off, sct, swt, groups):
    nc = bacc.Bacc("TRN2", target_bir_lowering=False, debug=False)

    emb_d = nc.dram_tensor("emb", [R, 128 * D], fp16, kind="ExternalInput").ap()
    span0_d = nc.dram_tensor("span0", [128, R, 128], fp16, kind="ExternalInput").ap()
    if sct:
        suf_d = nc.dram_tensor("suf", [sct, D], fp16, kind="ExternalInput").ap()
        spanp_d = nc.dram_tensor("spanp", [sct, 128], fp16, kind="ExternalInput").ap()
        outp_d = nc.dram_tensor("outp", [128, D], fp16, kind="ExternalOutput").ap()
    out_d = nc.dram_tensor("out", [R, 128, D], fp16, kind="ExternalOutput").ap()

    with tile.TileContext(nc) as tc:
        with (
            tc.tile_pool(name="emb", bufs=1) as embp,
            tc.tile_pool(name="span", bufs=1) as spanp,
            tc.tile_pool(name="outs", bufs=6) as outp,
            tc.tile_pool(name="psum", bufs=4, space="PSUM") as psump,
        ):
            emb_t = embp.tile([128, R, D], fp16, name="emb_t")
            span0_t = spanp.tile([128, R, 128], fp16, name="span0_t")
            if sct:
                suf_t = embp.tile([128, D], fp16, name="suf_t")
                spanp_t = spanp.tile([128, 128], fp16, name="spanp_t")
            psums = []

            def mm_pass(r, span_ap, emb_rows):
                ps = psump.tile([128, D], f32, name="ps")
                psums.append(ps)
                for n in range(D // NW):
                    f0 = n * NW
                    nc.tensor.matmul(
                        ps[:, f0 : f0 + NW],
                        span_ap,
                        emb_rows[:, f0 : f0 + NW],
                        start=True,
                        stop=True,
                    )

            # Emission order: consumers right after their producers' DMA
            # triggers; all inputs on the SP ring (~208GB/s observed; the
            # ACT ring is slower and keeps descriptor-gen time for stores),
            # in consumption order so the ring FIFO matches the matmul
            # order. 3 emb groups so late slots aren't gated on one big
            # transfer; the last group is just the final slot.
            nc.sync.dma_start(out=span0_t[:, 0, :], in_=span0_d[:, 0, :])
            nc.sync.dma_start(
                out=emb_t[: c0s[0], 0, :],
                in_=emb_d[0, : c0s[0] * D].rearrange("(p d) -> p d", d=D),
            )
            mm_pass(0, span0_t[: c0s[0], 0, :], emb_t[: c0s[0], 0, :])
            nc.sync.dma_start(out=span0_t[:, 1:, :], in_=span0_d[:, 1:, :])
            for gi, (lo, hi) in enumerate(groups):
                cg = max(c0s[lo:hi])
                nc.sync.dma_start(
                    out=emb_t[:cg, lo:hi, :],
                    in_=emb_d[lo:hi, : cg * D].rearrange("r (p d) -> p r d", d=D),
                )
                if gi == 0 and sct:
                    # pooled-pass inputs ride behind the first emb group:
                    # the bulk emb wire starts ~1us earlier and the pooled
                    # pass still runs well before the tail
                    nc.sync.dma_start(out=suf_t[:sct, :], in_=suf_d)
                    nc.sync.dma_start(out=spanp_t[:sct, :], in_=spanp_d)
            for r in range(1, R):
                mm_pass(r, span0_t[: c0s[r], r, :], emb_t[: c0s[r], r, :])
                if r == 3 and sct:
                    # pooled inputs land early; keep the pooled pass (and
                    # its copy + store) out of the tail
                    mm_pass("pool", spanp_t[:sct, :], suf_t[:sct, :])

            # copies + stores in completion order: ScalarE/VectorE alternate,
            # the pooled copy goes on ScalarE mid-stream. Full-tile [128, D]
            # copies + contiguous-row stores keep the store DMA on the fast
            # HW-DGE path (sliced APs fall back to ~70ns/descriptor inline
            # generation on the sequencer). VectorE can't trigger DMAs, so
            # its rows store via the SP ring (idle after the input stream).
            seq = [0, 1, 2, 3] + (["pool"] if sct else []) + list(range(4, R))
            keys = [0] + list(range(1, 4)) + (["pool"] if sct else []) + list(range(4, R))
            psum_of = dict(zip(keys, psums))
            for j, r in enumerate(seq):
                ps = psum_of[r]
                ot = outp.tile([128, D], fp16, name="ot")
                dst = outp_d if r == "pool" else out_d[r]
                if j % 2 == 0:
                    nc.scalar.copy(ot[:], ps[:])
                    nc.scalar.dma_start(out=dst, in_=ot[:])
                else:
                    nc.vector.tensor_copy(ot[:], ps[:])
                    nc.sync.dma_start(out=dst, in_=ot[:])

    nc.compile()
    return nc


_PROGRAM_CACHE = {}


def kernel(bert_embedding, x_bert_offset, x_mask, trace=False):
    global LAST_RESULTS
    assert bert_embedding.shape == (B, S, D), bert_embedding.shape
    rows = pack_rows(x_bert_offset, x_mask)
    assert max(g["nv"] for g in rows) <= 128, "over 128 valid words per row"
    assert max(g["sc"] for g in rows) <= 128 and max(g["sw"] for g in rows) <= 128
    perm, c0s, nps, scs, sws = assign_slots(rows)
    assert sum(scs) <= 128 and sum(sws) <= 128, (
        f"pooled suffix overflow: {sum(scs)} positions, {sum(sws)} words"
    )
    sc_off = tuple(int(x) for x in np.cumsum((0,) + scs[:-1]))
    sw_off = tuple(int(x) for x in np.cumsum((0,) + sws[:-1]))
    sct, swt = sum(scs), sum(sws)
    groups = ((1, 4), (4, 7), (7, 8))

    key = (c0s, nps, sc_off, sw_off, sct, swt, groups)
    if key not in _PROGRAM_CACHE:
        _PROGRAM_CACHE.clear()
        _PROGRAM_CACHE[key] = build_program(
            c0s, nps, sc_off, sw_off, sct, swt, groups
        )
    nc = _PROGRAM_CACHE[key]

    emb16 = np.asarray(bert_embedding).astype(np.float16)
    in_maps = []
    for c in range(N_CORES):
        emb_h = np.zeros((R, 128 * D), np.float16)
        span0_h = np.zeros((128, R, 128), np.float16)
        suf_h = np.zeros((max(sct, 1), D), np.float16)
        spanp_h = np.zeros((max(sct, 1), 128), np.float16)
        for r in range(R):
            b = perm[c][r]
            g = rows[b]
            packed = emb16[b, g["ci"]]  # [cov, D]
            emb_h[r, : g["bp"] * D] = packed[: g["bp"]].ravel()
            # prefix span: words [0, w1) x positions [0, bp)
            p = np.arange(g["bp"])
            w1 = g["w1"]
            m = (p[:, None] >= g["stp"][None, :w1]) & (
                p[:, None] < g["edp"][None, :w1]
            )
            span0_h[: g["bp"], r, :w1] = m * g["scale"][None, :w1]
            if g["sc"]:
                o = sc_off[r]
                suf_h[o : o + g["sc"]] = packed[g["bp"] :]
                # suffix span block: positions [bp, cov) x words [w1, nv)
                p = np.arange(g["bp"], g["cov"])
                m = (p[:, None] >= g["stp"][None, w1:]) & (
                    p[:, None] < g["edp"][None, w1:]
                )
                spanp_h[o : o + g["sc"], sw_off[r] : sw_off[r] + g["sw"]] = (
                    m * g["scale"][None, w1:]
                )
        m = {"emb": emb_h, "span0": span0_h}
        if sct:
            m["suf"] = suf_h
            m["spanp"] = spanp_h
        in_maps.append(m)

    res = run_bass_kernel_spmd(nc, in_maps, list(range(N_CORES)), trace=trace)
    LAST_RESULTS = res
    out = np.zeros((B, W, D), np.float32)
    for c in range(N_CORES):
        packed = res.results[c]["out"]
        poold = res.results[c]["outp"] if sct else None
        for r in range(R):
            b = perm[c][r]
            g = rows[b]
            out[b, g["idx"][: g["w1"]]] = packed[r, : g["w1"]]
            if g["sw"]:
                o = sw_off[r]
                out[b, g["idx"][g["w1"] :]] = poold[o : o + g["sw"]]
    return out
